# revision 25
# baseline (speedup 1.0000x reference)
"""Trainium2 Bass kernel for a dense transformer block (nn_Block_30520037605534).

Contract: kernel(**inputs) takes FULL unsharded fp32 inputs, returns FULL output.

v4 (8 cores, SPMD). Key changes vs v3 (469us):
  - fp8e4(m3) DoubleRow matmuls (0.5 cyc/row) for QKV, attention PV, and
    proj. Weights host-scaled into fp8 range (wq x64*SCALE, wk/wv/wproj
    x32); scales folded exactly: exp(in/2048 - 2) on ACT undoes q*k
    scaling, the v-ones column is 32.0 so PV normalization cancels wv's
    x32, proj's x32 is undone by a 1/32 scalar in the residual STT.
    FFN stays bf16 (fp8 there measured 3.1e-2 rel err > 2e-2 budget;
    the attention-path fp8 total sims at 7.9e-3).
  - Attention probs in fp8 via exp bias -2 (max prob ~e^3.5=33 < 240).
    PV over a PAIR of key blocks in one DoubleRow matmul.
  - Score matmuls for the two local heads emitted adjacently: K=64 row
    tiles land in different row-groups (auto tile_position from base
    partition 0/64) so the PE runs them concurrently.
  - Attention is then ACT(exp)-bound (~1.7us per key-block-pair vs
    ~1.3us cold PE), so the HAM half-clock state no longer matters.
  - LN2 mean handled by pre-subtracting a broadcast mu*rstd (2 DVE
    passes) instead of augmented contraction rows: no more K=2 aug
    matmuls on the PE; the beta/b_ff1 row rides the ReLU bias.
  - FFN weights (w1 8MB, w2 8MB, wproj 1MB) fully SBUF-resident,
    prefetched during attention on sync/vector/gpsimd queues: the FFN
    phase has zero weight DMA and runs dense back-to-back (HAM warm).
  - proj+LN2+FF1+FF2 run per batch-half: half A (batch 0 tokens) only
    needs the first AllToAll, so the second AllToAll's ~26us firmware
    latency hides entirely under half A's FFN.
  - bproj folded into xloc host-side; b_ff2 stays as the FF2 STT scalar.
"""

import os
from contextlib import ExitStack

import numpy as np

# ---- problem dims (hardcoded) ----
B, T, C, H, HS = 2, 2048, 1024, 16, 64
FF = 4 * C
N_CORES = 8
H_LOC = H // N_CORES          # 2 heads per core
EPS = 1e-5
SCALE = HS ** -0.5            # 1/8
SQ, SK, SV, SP = 64.0, 32.0, 32.0, 32.0   # fp8 range scales
EXP_BIAS = -2.0

_cache = {}


def _build(TT=T):
    import concourse.bass as bass
    import concourse.mybir as mybir
    import concourse.tile as tile
    from concourse import bacc
    from concourse.masks import make_identity

    f32 = mybir.dt.float32
    f32r = mybir.dt.float32r
    bf16 = mybir.dt.bfloat16
    f16 = mybir.dt.float16
    fp8 = mybir.dt.float8e4
    BT = B * TT
    TOK = BT // N_CORES         # tokens per core in data-parallel phases
    TOKH = TOK // 2             # tokens per core per batch
    NCH = BT // 512             # 512-token chunks (phase 1)
    NPB = C // 128              # 8 feature blocks
    NKB = TT // 128             # key blocks per batch (16)
    NPAIR = NKB // 2            # key-block pairs per batch (8)
    NQC = TT // 512             # query chunks per batch (4)
    NHB = FF // 128             # 32 hidden blocks
    AOp = mybir.AluOpType
    ACT = mybir.ActivationFunctionType
    DR = mybir.MatmulPerfMode.DoubleRow

    nc = bacc.Bacc("TRN2", target_bir_lowering=False, debug=False,
                   num_devices=N_CORES)

    _lp = ExitStack()
    _lp.enter_context(nc.allow_low_precision(
        "fp8 attention path + bf16 FFN; rel-err budget is 2e-2"))

    def mmr(out, lhsT, rhs, **kw):
        nc.tensor.matmul(out, lhsT.bitcast(f32r), rhs.bitcast(f32r), **kw)

    mm = nc.tensor.matmul

    def mmdr(out, lhsT, rhs, **kw):
        nc.tensor.matmul(out, lhsT, rhs, perf_mode=DR, **kw)

    # ---- DRAM I/O ----
    # All weight/const tensors arrive HOST-PRE-SHUFFLED into their SBUF
    # layout (partition dim first, per-partition rows contiguous) so every
    # DMA descriptor is a large contiguous run -- the naive
    # "(a p) -> p a" rearrange DMAs generate thousands of 4-128B
    # descriptors and took ~60us of queue time at kernel start.
    xt_d = nc.dram_tensor("xt", [128, NCH, NPB, 512], fp8,
                          kind="ExternalInput")                            # h1^T shuffled
    xloc_d = nc.dram_tensor("xloc", [C, TOK], bf16, kind="ExternalInput")  # (x+bproj)^T slice
    wq_d = nc.dram_tensor("wq", [128, NPB, 128], fp8, kind="ExternalInput")   # x SCALE*SQ
    wk_d = nc.dram_tensor("wk", [128, NPB, 128], fp8, kind="ExternalInput")   # x SK
    wv_d = nc.dram_tensor("wv", [128, NPB, 128], fp8, kind="ExternalInput")   # x SV
    wproj_d = nc.dram_tensor("wproj", [128, NPB, C], fp8, kind="ExternalInput")  # x SP
    wff1_d = nc.dram_tensor("wff1", [128, NPB, FF], bf16, kind="ExternalInput")  # g2-folded
    fbias_d = nc.dram_tensor("fbias", [128, NHB], f32, kind="ExternalInput")    # b2@W1+b_ff1
    wff2_d = nc.dram_tensor("wff2", [128, NHB, C], bf16, kind="ExternalInput")
    bff2_d = nc.dram_tensor("bff2", [128, NPB], f32, kind="ExternalInput")
    out_d = nc.dram_tensor("out", [C, TOK], bf16, kind="ExternalOutput")

    with tile.TileContext(nc) as tc:
        with (
            tc.tile_pool(name="const", bufs=1) as const,
            tc.tile_pool(name="dram", bufs=1, space="DRAM") as dram,
        ):
            # ---- small weights / constants resident in SBUF ----
            wq_t = const.tile([128, NPB, 128], fp8)
            wk_t = const.tile([128, NPB, 128], fp8)
            wv_t = const.tile([128, NPB, 128], fp8)
            for w_t, w_d in ((wq_t, wq_d), (wk_t, wk_d), (wv_t, wv_d)):
                nc.sync.dma_start(w_t[:], w_d.ap())

            # big weights, loaded during attention (emitted after phase 1)
            wproj_t = const.tile([128, NPB, C], fp8)
            w1_t = const.tile([128, NPB, FF], bf16)
            w2_t = const.tile([128, NHB, C], bf16)
            fbias_t = const.tile([128, NHB], f32)
            bff2_t = const.tile([128, NPB], f32)
            nc.gpsimd.dma_start(fbias_t[:], fbias_d.ap())
            nc.gpsimd.dma_start(bff2_t[:], bff2_d.ap())

            ones_colf = const.tile([128, 1], f32)
            nc.vector.memset(ones_colf[:], 1.0)
            ones_col_fr = const.tile([128, 1], f32r)
            nc.vector.tensor_copy(ones_col_fr[:], ones_colf[:])
            ones_rowf = const.tile([1, 128], f32)
            nc.vector.memset(ones_rowf[:], 1.0)
            ones_row_fr = const.tile([1, 128], f32r)
            nc.vector.tensor_copy(ones_row_fr[:], ones_rowf[:])
            inv_sp_col = const.tile([128, 1], f32)
            nc.vector.memset(inv_sp_col[:], 1.0 / SP)
            eps_col = const.tile([128, 1], f32)
            nc.vector.memset(eps_col[:], EPS)
            ebias_col = const.tile([128, 1], f32)
            nc.vector.memset(ebias_col[:], EXP_BIAS)
            maskNeg = const.tile([128, 128], f32)
            nc.gpsimd.memset(maskNeg[:], 0.0)
            nc.gpsimd.affine_select(
                out=maskNeg[:], in_=maskNeg[:],
                compare_op=AOp.is_ge, fill=-1.0e6,
                base=0, pattern=[[1, 128]], channel_multiplier=-1)
            identF = const.tile([128, 128], f32)
            make_identity(nc, identF[:])
            ident16 = const.tile([128, 128], f16)
            nc.vector.tensor_copy(ident16[:], identF[:])

            # persistent stores (freed after attention)
            es_qkv = ExitStack()
            store_qk = es_qkv.enter_context(tc.tile_pool(name="store_qk", bufs=1))
            store_v = es_qkv.enter_context(tc.tile_pool(name="store_v", bufs=1))
            qT_st = store_qk.tile([128, BT], bf16)
            kT_st = store_qk.tile([128, BT], bf16)
            # v^T per (batch-pair, head): [keys=128, pair, hh, j, 80]
            # col 64 = 32.0 (row-sum column; 32 = SV so normalization
            # cancels wv's scale); cols 65.. are pad for 16B Ko stride.
            v_st = store_v.tile([128, B * NPAIR, H_LOC, 2, 80], fp8)
            nc.vector.memset(v_st[:, :, :, :, 64:65], SV)

            # ======== Phase 1: QKV (fp8 DoubleRow) + v transposed ====
            with (
                nc.named_scope("ph1"),
                tc.tile_pool(name="p1x", bufs=2) as p1x,
                tc.tile_pool(name="ps_q", bufs=2, space="PSUM") as ps_q,
                tc.tile_pool(name="ps_k", bufs=2, space="PSUM") as ps_k,
                tc.tile_pool(name="ps_v", bufs=2, space="PSUM") as ps_v,
                tc.tile_pool(name="ps_vt", bufs=2, space="PSUM") as ps_vt,
            ):
                for tch in range(NCH):
                    t0 = tch * 512
                    xt = p1x.tile([128, NPB, 512], fp8, tag="xt")
                    eng = (nc.sync, nc.gpsimd, nc.scalar)[tch % 3]
                    eng.dma_start(xt[:], xt_d.ap()[:, tch, :, :])
                    q_ps = ps_q.tile([128, 512], f32, tag="q")
                    k_ps = ps_k.tile([128, 512], f32, tag="k")
                    v_ps = ps_v.tile([128, 512], f32, tag="v")
                    for j in range(NPB // 2):
                        sl = slice(2 * j, 2 * j + 2)
                        st, sp_ = (j == 0), (j == NPB // 2 - 1)
                        mmdr(q_ps[:], wq_t[:, sl, :], xt[:, sl, :],
                             start=st, stop=sp_)
                        mmdr(k_ps[:], wk_t[:, sl, :], xt[:, sl, :],
                             start=st, stop=sp_)
                        mmdr(v_ps[:], wv_t[:, sl, :], xt[:, sl, :],
                             start=st, stop=sp_)
                    nc.vector.tensor_copy(qT_st[:, t0:t0 + 512], q_ps[:])
                    nc.vector.tensor_copy(kT_st[:, t0:t0 + 512], k_ps[:])
                    v_row = p1x.tile([128, 512], f16, tag="vrow")
                    nc.vector.tensor_copy(v_row[:], v_ps[:])
                    for sb in range(4):
                        kbg = (t0 + sb * 128) // 128
                        b_, kb = kbg // NKB, kbg % NKB
                        pr, jj = kb // 2, kb % 2
                        # one full-width transpose covers both heads
                        vtr = ps_vt.tile([128, 128], f16, tag="vtr")
                        nc.tensor.transpose(
                            vtr[:],
                            v_row[:, sb * 128:(sb + 1) * 128],
                            ident16[:])
                        for hh in range(H_LOC):
                            nc.vector.tensor_copy(
                                v_st[:, b_ * NPAIR + pr, hh, jj, 0:64],
                                vtr[:, hh * 64:hh * 64 + 64])

            # prefetch big weights during attention (not needed until proj/FFN)
            # Gate the big weight prefetches behind phase 1's last xt
            # chunk: a 1-element DVE copy into each weight tile creates a
            # WAW dep, so their transfers can't start until the xt feed is
            # done competing for HBM bandwidth (phase 1 is HBM-bound).
            for wt in (wproj_t, w1_t, w2_t):
                nc.vector.tensor_copy(wt[0:1, 0, 0:1], qT_st[0:1, BT - 1:BT])
            nc.sync.dma_start(wproj_t[:], wproj_d.ap())
            nc.scalar.dma_start(w1_t[:, 0:4, :], wff1_d.ap()[:, 0:4, :])
            nc.scalar.dma_start(w1_t[:, 4:8, :], wff1_d.ap()[:, 4:8, :])
            nc.gpsimd.dma_start(w2_t[:], wff2_d.ap())
            # xl prefetch during attention (pure input, off the proj path)
            ao_loc = const.tile([128, NPB, TOK], fp8, tag="aoloc")
            xl = const.tile([128, NPB, TOK], bf16, tag="xl")
            for pb in range(NPB):
                eng = (nc.sync, nc.scalar)[pb % 2]
                eng.dma_start(xl[:, pb, :],
                              xloc_d.ap()[pb * 128:(pb + 1) * 128, :])

            # ======== Phase 2: causal attention, fp8 probs, DR PV ====
            a2a_in = [dram.tile([N_CORES, 128, TOKH], fp8, tag=f"a2a_in{b}",
                                name=f"a2a_in{b}")
                      for b in range(B)]
            a2a_out = [dram.tile([N_CORES, 128, TOKH], fp8, tag=f"a2a_out{b}",
                                 name=f"a2a_out{b}")
                       for b in range(B)]
            with (
                nc.named_scope("attn"),
                tc.tile_pool(name="p2e0", bufs=2) as p2e0,
                tc.tile_pool(name="p2e1", bufs=2) as p2e1,
                tc.tile_pool(name="p2s", bufs=2) as p2s,
                tc.tile_pool(name="ps_sc0", bufs=1, space="PSUM") as ps_sc0,
                tc.tile_pool(name="ps_sc1", bufs=1, space="PSUM") as ps_sc1,
                tc.tile_pool(name="ps_o", bufs=2, space="PSUM") as ps_o,
            ):
                ps_sc = (ps_sc0, ps_sc1)
                p2e = (p2e0, p2e1)

                def emit_pv(pv):
                    (b, pr, npr, fs0, ex_t, o_ps) = pv
                    for hh in range(H_LOC):
                        mmdr(o_ps[hh][0:65, fs0:512],
                             v_st[:, b * NPAIR + pr, hh, :, 0:65],
                             ex_t[hh][:, :, fs0:512],
                             start=(pr == 0), stop=(pr == npr - 1))

                def emit_norm_r(nrm):
                    (b, qo, o_ps) = nrm
                    r_row = []
                    for hh in range(H_LOC):
                        r = p2s.tile([1, 512], f32r, tag=f"r{hh}",
                                     name=f"r{hh}")
                        nc.vector.tensor_copy(r[:], o_ps[hh][64:65, :])
                        r_row.append(r)
                    return (b, qo, o_ps, r_row)

                def emit_norm2(nrm2):
                    (b, qo, o_ps, r_row) = nrm2
                    rb_ps = [ps_sc1.tile([128, 2, 512], f32, tag="sc1",
                                         name=f"rb{hh}")
                             for hh in range(H_LOC)]
                    for hh in range(H_LOC):
                        mmr(rb_ps[hh][:, 0, :], ones_row_fr[:], r_row[hh][:],
                            start=True, stop=True)
                    for hh in range(H_LOC):
                        hp = hh * 64
                        rb_sb = p2s.tile([64, 512], f32, tag=f"rbsb{hh}",
                                         name=f"rbsb{hh}")
                        nc.vector.reciprocal_approx_fast(
                            rb_sb[:], rb_ps[hh][0:64, 0, :])
                        ao_t = p2s.tile([64, 512], fp8, tag=f"ao{hh}",
                                        name=f"ao{hh}")
                        nc.vector.tensor_mul(ao_t[:], o_ps[hh][0:64, :],
                                             rb_sb[:])
                        for jj in range(2):
                            a0 = qo // TOKH + jj
                            nc.sync.dma_start(
                                a2a_in[b][a0, hp:hp + 64, :],
                                ao_t[:, jj * TOKH:(jj + 1) * TOKH])

                                # software pipeline: PV trails its sc/exp by one pair-unit
                # and the per-qc norm trails into the next qc, so the ACT
                # exp stream never waits on the PE and vice versa.
                pend_pv = None
                pend_norm = None
                pend_norm2 = None
                for b in range(B):
                    for qc in range(NQC):
                        qo = qc * 512
                        nkb = qo // 128 + 4
                        npr = nkb // 2
                        o_ps = [ps_o.tile([128, 512], f32, tag=f"o{hh}",
                                          name=f"o{hh}")
                                for hh in range(H_LOC)]
                        for pr in range(npr):
                            kb0 = 2 * pr
                            diag = kb0 >= nkb - 4
                            fs0 = max(0, kb0 * 128 - qo) if diag else 0
                            sc_t = [ps_sc[hh].tile([128, 2, 512], f32,
                                                   tag=f"sc{hh}",
                                                   name=f"sc{hh}")
                                    for hh in range(H_LOC)]
                            for j in range(2):
                                kb = kb0 + j
                                fsj = max(0, kb * 128 - qo) if diag else 0
                                for hh in range(H_LOC):
                                    hp = hh * 64
                                    mm(sc_t[hh][:, j, fsj:512],
                                       kT_st[hp:hp + 64,
                                             b * TT + kb * 128:
                                             b * TT + (kb + 1) * 128],
                                       qT_st[hp:hp + 64,
                                             b * TT + qo + fsj:
                                             b * TT + qo + 512],
                                       start=True, stop=True)
                            ex_t = []
                            for hh in range(H_LOC):
                                ex = p2e[hh].tile([128, 2, 512], fp8,
                                                  tag=f"ex{hh}",
                                                  name=f"ex{hh}")
                                if not diag:
                                    nc.scalar.activation(
                                        ex[:], sc_t[hh][:], ACT.Exp,
                                        bias=ebias_col[:, 0:1],
                                        scale=1.0 / (SQ * SK))
                                else:
                                    for j in range(2):
                                        dj = (kb0 + j) * 128 - qo
                                        if dj > fs0:
                                            nc.vector.memset(
                                                ex[:, j, fs0:dj], 0.0)
                                        nc.vector.tensor_add(
                                            sc_t[hh][:, j, dj:dj + 128],
                                            sc_t[hh][:, j, dj:dj + 128],
                                            maskNeg[:])
                                        nc.scalar.activation(
                                            ex[:, j, dj:512],
                                            sc_t[hh][:, j, dj:512], ACT.Exp,
                                            bias=ebias_col[:, 0:1],
                                            scale=1.0 / (SQ * SK))
                                ex_t.append(ex)
                            if pend_pv is not None:
                                emit_pv(pend_pv)
                            if pend_norm2 is not None:
                                emit_norm2(pend_norm2)
                                pend_norm2 = None
                            elif pend_norm is not None:
                                pend_norm2 = emit_norm_r(pend_norm)
                                pend_norm = None
                            pend_pv = (b, pr, npr, fs0, ex_t, o_ps)
                        pend_norm = (b, qo, o_ps)
                    # flush before firing this batch's AllToAll
                    emit_pv(pend_pv)
                    pend_pv = None
                    if pend_norm2 is not None:
                        emit_norm2(pend_norm2)
                        pend_norm2 = None
                    emit_norm2(emit_norm_r(pend_norm))
                    pend_norm = None
                    nc.gpsimd.collective_compute(
                        "AllToAll", mybir.AluOpType.bypass,
                        replica_groups=[list(range(N_CORES))],
                        ins=[a2a_in[b].opt()], outs=[a2a_out[b].opt()])

            es_qkv.close()   # free q/k/v stores

            # ======== Phase 3: per-half proj + LN2 + FFN ====
            with (
                nc.named_scope("ffn"),
                tc.tile_pool(name="p3a", bufs=1) as p3a,
                tc.tile_pool(name="p3t", bufs=2) as p3t,
                tc.tile_pool(name="p3s", bufs=1) as p3s,
                tc.tile_pool(name="ps_pj", bufs=2, space="PSUM") as ps_pj,
                tc.tile_pool(name="ps_st", bufs=1, space="PSUM") as ps_st,
                tc.tile_pool(name="ps_f1", bufs=2, space="PSUM") as ps_f1,
                tc.tile_pool(name="ps_f2", bufs=2, space="PSUM") as ps_f2,
            ):
                y = p3a.tile([128, NPB, TOK], f32r, tag="y")
                yp = p3a.tile([128, NPB, TOK], bf16, tag="yp")

                for bb in range(B):
                    hs_ = slice(bb * TOKH, (bb + 1) * TOKH)
                    for a in range(N_CORES):
                        if bb == 0:
                            eng = (nc.sync, nc.scalar)[a % 2]
                        else:
                            eng = nc.gpsimd
                        eng.dma_start(ao_loc[:, a, hs_], a2a_out[bb][a, :, :])
                    # ---- proj (fp8 DR) + residual ----
                    for co in range(NPB):
                        pj_ps = ps_f1.tile([128, TOKH], f32, tag="f1",
                                           name="pj_ps")
                        for j in range(NPB // 2):
                            sl = slice(2 * j, 2 * j + 2)
                            mmdr(pj_ps[:],
                                 wproj_t[:, sl, co * 128:(co + 1) * 128],
                                 ao_loc[:, sl, hs_],
                                 start=(j == 0), stop=(j == NPB // 2 - 1))
                        # y = pj/SP + (x + bproj)
                        nc.vector.scalar_tensor_tensor(
                            out=y[:, co, hs_], in0=pj_ps[:],
                            scalar=inv_sp_col[:, 0:1],
                            in1=xl[:, co, hs_], op0=AOp.mult, op1=AOp.add)
                    # ---- LN2 stats ----
                    s_ps = ps_st.tile([1, TOKH], f32, tag="s")
                    s2_ps = ps_st.tile([1, TOKH], f32, tag="s2")
                    for pb in range(NPB):
                        sq = p3t.tile([128, TOKH], f32r, tag="sq")
                        nc.vector.tensor_mul(sq[:], y[:, pb, hs_],
                                             y[:, pb, hs_])
                        mmr(s_ps[:], ones_col_fr[:], y[:, pb, hs_],
                            start=(pb == 0), stop=(pb == NPB - 1))
                        mmr(s2_ps[:], ones_col_fr[:], sq[:],
                            start=(pb == 0), stop=(pb == NPB - 1))
                    mu = p3s.tile([1, TOKH], f32r, tag="mu")
                    e2 = p3s.tile([1, TOKH], f32r, tag="e2")
                    nc.scalar.mul(mu[:], s_ps[:], 1.0 / C)
                    nc.scalar.mul(e2[:], s2_ps[:], 1.0 / C)
                    var = p3s.tile([1, TOKH], f32r, tag="var")
                    nc.vector.tensor_mul(var[:], mu[:], mu[:])
                    nc.vector.tensor_sub(var[:], e2[:], var[:])
                    R2_ps = ps_pj.tile([128, TOKH], f32, tag="pj",
                                       name="R2_ps")
                    mmr(R2_ps[:], ones_row_fr[:], var[:], start=True, stop=True)
                    R2_std = p3s.tile([128, TOKH], f32, tag="R2std")
                    nc.scalar.activation(R2_std[:], R2_ps[:], ACT.Sqrt,
                                         bias=eps_col[:])
                    R2_sb = p3s.tile([128, TOKH], f32, tag="R2sb")
                    nc.vector.reciprocal_approx_fast(R2_sb[:], R2_std[:])
                    # broadcast mu*rstd, then yp = y*rstd - bcast(mu*rstd)
                    mr = p3s.tile([1, TOKH], f32r, tag="mr")
                    nc.vector.tensor_mul(mr[:], mu[:], R2_sb[0:1, :])
                    MR_ps = ps_pj.tile([128, TOKH], f32, tag="pj", name="MR_ps")
                    mmr(MR_ps[:], ones_row_fr[:], mr[:], start=True, stop=True)
                    MR_sb = p3s.tile([128, TOKH], f32, tag="MRsb")
                    nc.vector.tensor_copy(MR_sb[:], MR_ps[:])
                    for pb in range(NPB):
                        t = p3t.tile([128, TOKH], f32r, tag="t")
                        nc.vector.tensor_mul(t[:], y[:, pb, hs_], R2_sb[:])
                        nc.vector.tensor_sub(yp[:, pb, hs_], t[:], MR_sb[:])
                    # ---- FF1 (+ReLU with fused bias row) ----
                    F = p3a.tile([128, NHB, TOKH], bf16, tag="F")
                    for hb in range(NHB):
                        f1_ps = ps_f1.tile([128, TOKH], f32, tag="f1")
                        for pb in range(NPB):
                            mm(f1_ps[:], w1_t[:, pb, hb * 128:(hb + 1) * 128],
                               yp[:, pb, hs_],
                               start=(pb == 0), stop=(pb == NPB - 1))
                        nc.scalar.activation(F[:, hb, :], f1_ps[:], ACT.Relu,
                                             bias=fbias_t[:, hb:hb + 1])
                    # ---- FF2 + residual ----
                    for co in range(NPB):
                        f2_ps = ps_f2.tile([128, TOKH], f32, tag="f2")
                        for hb in range(NHB):
                            mm(f2_ps[:], w2_t[:, hb, co * 128:(co + 1) * 128],
                               F[:, hb, :],
                               start=(hb == 0), stop=(hb == NHB - 1))
                        ob = p3t.tile([128, TOKH], bf16, tag="ob")
                        nc.vector.scalar_tensor_tensor(
                            out=ob[:], in0=f2_ps[:],
                            scalar=bff2_t[:, co:co + 1],
                            in1=y[:, co, hs_], op0=AOp.add, op1=AOp.add)
                        nc.gpsimd.dma_start(
                            out_d.ap()[co * 128:(co + 1) * 128, hs_], ob[:])

    nc.compile()
    return nc


def _make_in_maps(x, Wq, Wk, Wv, Wproj, bproj, g1, b1, g2, b2,
                  W_ff1, b_ff1, W_ff2, b_ff2, TT=T):
    import ml_dtypes
    bf16 = ml_dtypes.bfloat16
    fp8 = ml_dtypes.float8_e4m3
    BT = B * TT
    TOK = BT // N_CORES
    TOKH = TOK // 2
    NCH = BT // 512
    NPB = C // 128
    NHB = FF // 128
    f = np.float32

    def shuf(w, nblk):
        """[nblk*128, M] row-major -> [128, nblk, M] partition-major."""
        w = np.asarray(w)
        return np.ascontiguousarray(
            w.reshape(nblk, 128, w.shape[1]).transpose(1, 0, 2))

    x2d = np.asarray(x, f).reshape(BT, C)
    # LN1 applied on the host (pure function of the input x); the
    # (x + bproj) residual flows through the separate xloc input
    mu = x2d.mean(1, keepdims=True)
    rstd = 1.0 / np.sqrt(x2d.var(1, keepdims=True) + EPS)
    h1 = ((x2d - mu) * rstd * np.asarray(g1, f) + np.asarray(b1, f)).astype(f)
    # xt pre-shuffled: [128, NCH, NPB, 512], chunk-contiguous per partition
    xts = np.ascontiguousarray(
        h1.T.reshape(NPB, 128, NCH, 512).transpose(1, 2, 0, 3)).astype(fp8)
    xraw = np.ascontiguousarray(
        (x2d + np.asarray(bproj, f)[None, :]).T).astype(bf16)
    w1f = shuf((np.asarray(g2, f)[:, None]
                * np.asarray(W_ff1, f)).astype(bf16), NPB)
    fbias = np.ascontiguousarray(
        (np.asarray(b2, f) @ np.asarray(W_ff1, f)
         + np.asarray(b_ff1, f)).astype(f).reshape(NHB, 128).T)
    w2f = shuf(np.asarray(W_ff2, f).astype(bf16), NHB)
    wpj = shuf((np.asarray(Wproj, f) * SP).astype(fp8), NPB)
    bf2 = np.ascontiguousarray(
        np.asarray(b_ff2, f).reshape(NPB, 128).T)

    in_maps = []
    for c in range(N_CORES):
        h0 = c * H_LOC
        per_head = []
        for W, s_ in ((Wq, SCALE * SQ), (Wk, SK), (Wv, SV)):
            wl = np.ascontiguousarray(
                np.transpose(np.asarray(W, f)[h0:h0 + H_LOC], (1, 0, 2))
            ).reshape(C, H_LOC * HS) * s_
            per_head.append(shuf(wl.astype(fp8), NPB))
        # split-token ownership: core c owns tokens [TOKH*c, TOKH*(c+1))
        # of EACH batch (matches the per-batch AllToAlls)
        cols = np.concatenate([
            np.arange(TOKH * c, TOKH * (c + 1)),
            np.arange(TT + TOKH * c, TT + TOKH * (c + 1))])
        in_maps.append({
            "xt": xts,
            "xloc": np.ascontiguousarray(xraw[:, cols]),
            "wq": per_head[0], "wk": per_head[1], "wv": per_head[2],
            "wproj": wpj,
            "wff1": w1f,
            "fbias": fbias,
            "wff2": w2f,
            "bff2": bf2,
        })
    return in_maps


def _gather_out(shards, TT=T):
    """Assemble per-core [C, TOK] shards (split-token ownership) -> [C, BT]."""
    BT = B * TT
    TOK = BT // N_CORES
    TOKH = TOK // 2
    outT = np.empty((C, BT), np.float32)
    for c, sh in enumerate(shards):
        cols = np.concatenate([
            np.arange(TOKH * c, TOKH * (c + 1)),
            np.arange(TT + TOKH * c, TT + TOKH * (c + 1))])
        outT[:, cols] = sh
    return outT


def kernel(**inputs):
    from concourse.bass_utils import run_bass_kernel_spmd
    if "nc" not in _cache:
        _cache["nc"] = _build()
    nc = _cache["nc"]
    in_maps = _make_in_maps(**inputs)
    res = run_bass_kernel_spmd(nc, in_maps, list(range(N_CORES)),
                               trace=bool(int(os.environ.get("KERNEL_TRACE", "0"))))
    _cache["last_result"] = res
    shards = [np.asarray(res.results[c]["out"], np.float32)
              for c in range(N_CORES)]                      # each [C, TOK]
    outT = _gather_out(shards)
    return np.ascontiguousarray(outT.T).reshape(B, T, C)


# revision 26
# speedup vs baseline: 1.0459x; 1.0459x over previous
"""Trainium2 Bass kernel for a dense transformer block (nn_Block_30520037605534).

Contract: kernel(**inputs) takes FULL unsharded fp32 inputs, returns FULL output.

v4 (8 cores, SPMD). Key changes vs v3 (469us):
  - fp8e4(m3) DoubleRow matmuls (0.5 cyc/row) for QKV, attention PV, and
    proj. Weights host-scaled into fp8 range (wq x64*SCALE, wk/wv/wproj
    x32); scales folded exactly: exp(in/2048 - 2) on ACT undoes q*k
    scaling, the v-ones column is 32.0 so PV normalization cancels wv's
    x32, proj's x32 is undone by a 1/32 scalar in the residual STT.
    FFN stays bf16 (fp8 there measured 3.1e-2 rel err > 2e-2 budget;
    the attention-path fp8 total sims at 7.9e-3).
  - Attention probs in fp8 via exp bias -2 (max prob ~e^3.5=33 < 240).
    PV over a PAIR of key blocks in one DoubleRow matmul.
  - Score matmuls for the two local heads emitted adjacently: K=64 row
    tiles land in different row-groups (auto tile_position from base
    partition 0/64) so the PE runs them concurrently.
  - Attention is then ACT(exp)-bound (~1.7us per key-block-pair vs
    ~1.3us cold PE), so the HAM half-clock state no longer matters.
  - LN2 mean handled by pre-subtracting a broadcast mu*rstd (2 DVE
    passes) instead of augmented contraction rows: no more K=2 aug
    matmuls on the PE; the beta/b_ff1 row rides the ReLU bias.
  - FFN weights (w1 8MB, w2 8MB, wproj 1MB) fully SBUF-resident,
    prefetched during attention on sync/vector/gpsimd queues: the FFN
    phase has zero weight DMA and runs dense back-to-back (HAM warm).
  - proj+LN2+FF1+FF2 run per batch-half: half A (batch 0 tokens) only
    needs the first AllToAll, so the second AllToAll's ~26us firmware
    latency hides entirely under half A's FFN.
  - bproj folded into xloc host-side; b_ff2 stays as the FF2 STT scalar.
"""

import os
from contextlib import ExitStack

import numpy as np

# ---- problem dims (hardcoded) ----
B, T, C, H, HS = 2, 2048, 1024, 16, 64
FF = 4 * C
N_CORES = 8
H_LOC = H // N_CORES          # 2 heads per core
EPS = 1e-5
SCALE = HS ** -0.5            # 1/8
SQ, SK, SV, SP = 64.0, 32.0, 32.0, 32.0   # fp8 range scales
EXP_BIAS = -2.0

_cache = {}


def _build(TT=T):
    import concourse.bass as bass
    import concourse.mybir as mybir
    import concourse.tile as tile
    from concourse import bacc
    from concourse.masks import make_identity

    f32 = mybir.dt.float32
    f32r = mybir.dt.float32r
    bf16 = mybir.dt.bfloat16
    f16 = mybir.dt.float16
    fp8 = mybir.dt.float8e4
    BT = B * TT
    TOK = BT // N_CORES         # tokens per core in data-parallel phases
    TOKH = TOK // 2             # tokens per core per batch
    NCH = BT // 512             # 512-token chunks (phase 1)
    NPB = C // 128              # 8 feature blocks
    NKB = TT // 128             # key blocks per batch (16)
    NPAIR = NKB // 2            # key-block pairs per batch (8)
    NQC = TT // 512             # query chunks per batch (4)
    NHB = FF // 128             # 32 hidden blocks
    AOp = mybir.AluOpType
    ACT = mybir.ActivationFunctionType
    DR = mybir.MatmulPerfMode.DoubleRow

    nc = bacc.Bacc("TRN2", target_bir_lowering=False, debug=False,
                   num_devices=N_CORES)

    _lp = ExitStack()
    _lp.enter_context(nc.allow_low_precision(
        "fp8 attention path + bf16 FFN; rel-err budget is 2e-2"))

    def mmr(out, lhsT, rhs, **kw):
        nc.tensor.matmul(out, lhsT.bitcast(f32r), rhs.bitcast(f32r), **kw)

    mm = nc.tensor.matmul

    def mmdr(out, lhsT, rhs, **kw):
        nc.tensor.matmul(out, lhsT, rhs, perf_mode=DR, **kw)

    # ---- DRAM I/O ----
    # All weight/const tensors arrive HOST-PRE-SHUFFLED into their SBUF
    # layout (partition dim first, per-partition rows contiguous) so every
    # DMA descriptor is a large contiguous run -- the naive
    # "(a p) -> p a" rearrange DMAs generate thousands of 4-128B
    # descriptors and took ~60us of queue time at kernel start.
    xt_d = nc.dram_tensor("xt", [128, NCH, NPB, 512], fp8,
                          kind="ExternalInput")                            # h1^T shuffled
    xloc_d = nc.dram_tensor("xloc", [C, TOK], bf16, kind="ExternalInput")  # (x+bproj)^T slice
    wq_d = nc.dram_tensor("wq", [128, NPB, 128], fp8, kind="ExternalInput")   # x SCALE*SQ
    wk_d = nc.dram_tensor("wk", [128, NPB, 128], fp8, kind="ExternalInput")   # x SK
    wv_d = nc.dram_tensor("wv", [128, NPB, 128], fp8, kind="ExternalInput")   # x SV
    wproj_d = nc.dram_tensor("wproj", [128, NPB, C], fp8, kind="ExternalInput")  # x SP
    wff1_d = nc.dram_tensor("wff1", [128, NPB, FF], bf16, kind="ExternalInput")  # g2-folded
    fbias_d = nc.dram_tensor("fbias", [128, NHB], f32, kind="ExternalInput")    # b2@W1+b_ff1
    wff2_d = nc.dram_tensor("wff2", [128, NHB, C], bf16, kind="ExternalInput")
    bff2_d = nc.dram_tensor("bff2", [128, NPB], f32, kind="ExternalInput")
    out_d = nc.dram_tensor("out", [C, TOK], bf16, kind="ExternalOutput")

    with tile.TileContext(nc) as tc:
        with (
            tc.tile_pool(name="const", bufs=1) as const,
            tc.tile_pool(name="dram", bufs=1, space="DRAM") as dram,
        ):
            # ---- small weights / constants resident in SBUF ----
            wq_t = const.tile([128, NPB, 128], fp8)
            wk_t = const.tile([128, NPB, 128], fp8)
            wv_t = const.tile([128, NPB, 128], fp8)
            for w_t, w_d in ((wq_t, wq_d), (wk_t, wk_d), (wv_t, wv_d)):
                nc.sync.dma_start(w_t[:], w_d.ap())

            # big weights, loaded during attention (emitted after phase 1)
            wproj_t = const.tile([128, NPB, C], fp8)
            w1_t = const.tile([128, NPB, FF], bf16)
            w2_t = const.tile([128, NHB, C], bf16)
            fbias_t = const.tile([128, NHB], f32)
            bff2_t = const.tile([128, NPB], f32)
            nc.gpsimd.dma_start(fbias_t[:], fbias_d.ap())
            nc.gpsimd.dma_start(bff2_t[:], bff2_d.ap())

            ones_colf = const.tile([128, 1], f32)
            nc.vector.memset(ones_colf[:], 1.0)
            ones_col_fr = const.tile([128, 1], f32r)
            nc.vector.tensor_copy(ones_col_fr[:], ones_colf[:])
            ones_rowf = const.tile([1, 128], f32)
            nc.vector.memset(ones_rowf[:], 1.0)
            ones_row_fr = const.tile([1, 128], f32r)
            nc.vector.tensor_copy(ones_row_fr[:], ones_rowf[:])
            inv_sp_col = const.tile([128, 1], f32)
            nc.vector.memset(inv_sp_col[:], 1.0 / SP)
            eps_col = const.tile([128, 1], f32)
            nc.vector.memset(eps_col[:], EPS)
            ebias_col = const.tile([128, 1], f32)
            nc.vector.memset(ebias_col[:], EXP_BIAS)
            maskNeg = const.tile([128, 128], f32)
            nc.gpsimd.memset(maskNeg[:], 0.0)
            nc.gpsimd.affine_select(
                out=maskNeg[:], in_=maskNeg[:],
                compare_op=AOp.is_ge, fill=-1.0e6,
                base=0, pattern=[[1, 128]], channel_multiplier=-1)
            identF = const.tile([128, 128], f32)
            make_identity(nc, identF[:])
            ident16 = const.tile([128, 128], f16)
            nc.vector.tensor_copy(ident16[:], identF[:])

            # persistent stores (freed after attention)
            es_qkv = ExitStack()
            store_qk = es_qkv.enter_context(tc.tile_pool(name="store_qk", bufs=1))
            store_v = es_qkv.enter_context(tc.tile_pool(name="store_v", bufs=1))
            qT_st = store_qk.tile([128, BT], bf16)
            kT_st = store_qk.tile([128, BT], bf16)
            # v^T per (batch-pair, head): [keys=128, pair, hh, j, 80]
            # col 64 = 32.0 (row-sum column; 32 = SV so normalization
            # cancels wv's scale); cols 65.. are pad for 16B Ko stride.
            v_st = store_v.tile([128, B * NPAIR, H_LOC, 2, 80], fp8)
            nc.vector.memset(v_st[:, :, :, :, 64:65], SV)

            # ======== Phase 1: QKV (fp8 DoubleRow) + v transposed ====
            with (
                nc.named_scope("ph1"),
                tc.tile_pool(name="p1x", bufs=2) as p1x,
                tc.tile_pool(name="ps_q", bufs=2, space="PSUM") as ps_q,
                tc.tile_pool(name="ps_k", bufs=2, space="PSUM") as ps_k,
                tc.tile_pool(name="ps_v", bufs=2, space="PSUM") as ps_v,
                tc.tile_pool(name="ps_vt", bufs=2, space="PSUM") as ps_vt,
            ):
                for tch in range(NCH):
                    t0 = tch * 512
                    xt = p1x.tile([128, NPB, 512], fp8, tag="xt")
                    eng = (nc.sync, nc.gpsimd, nc.scalar)[tch % 3]
                    eng.dma_start(xt[:], xt_d.ap()[:, tch, :, :])
                    q_ps = ps_q.tile([128, 512], f32, tag="q")
                    k_ps = ps_k.tile([128, 512], f32, tag="k")
                    v_ps = ps_v.tile([128, 512], f32, tag="v")
                    for j in range(NPB // 2):
                        sl = slice(2 * j, 2 * j + 2)
                        st, sp_ = (j == 0), (j == NPB // 2 - 1)
                        mmdr(q_ps[:], wq_t[:, sl, :], xt[:, sl, :],
                             start=st, stop=sp_)
                        mmdr(k_ps[:], wk_t[:, sl, :], xt[:, sl, :],
                             start=st, stop=sp_)
                        mmdr(v_ps[:], wv_t[:, sl, :], xt[:, sl, :],
                             start=st, stop=sp_)
                    nc.vector.tensor_copy(qT_st[:, t0:t0 + 512], q_ps[:])
                    nc.vector.tensor_copy(kT_st[:, t0:t0 + 512], k_ps[:])
                    v_row = p1x.tile([128, 512], f16, tag="vrow")
                    nc.vector.tensor_copy(v_row[:], v_ps[:])
                    for sb in range(4):
                        kbg = (t0 + sb * 128) // 128
                        b_, kb = kbg // NKB, kbg % NKB
                        pr, jj = kb // 2, kb % 2
                        # one full-width transpose covers both heads
                        vtr = ps_vt.tile([128, 128], f16, tag="vtr")
                        nc.tensor.transpose(
                            vtr[:],
                            v_row[:, sb * 128:(sb + 1) * 128],
                            ident16[:])
                        for hh in range(H_LOC):
                            nc.vector.tensor_copy(
                                v_st[:, b_ * NPAIR + pr, hh, jj, 0:64],
                                vtr[:, hh * 64:hh * 64 + 64])

            # prefetch big weights during attention (not needed until proj/FFN)
            # Gate the big weight prefetches behind phase 1's last xt
            # chunk: a 1-element DVE copy into each weight tile creates a
            # WAW dep, so their transfers can't start until the xt feed is
            # done competing for HBM bandwidth (phase 1 is HBM-bound).
            for wt in (wproj_t, w1_t, w2_t):
                nc.vector.tensor_copy(wt[0:1, 0, 0:1], qT_st[0:1, BT - 1:BT])
            nc.sync.dma_start(wproj_t[:], wproj_d.ap())
            nc.scalar.dma_start(w1_t[:, 0:4, :], wff1_d.ap()[:, 0:4, :])
            nc.scalar.dma_start(w1_t[:, 4:8, :], wff1_d.ap()[:, 4:8, :])
            nc.gpsimd.dma_start(w2_t[:], wff2_d.ap())
            # xl prefetch during attention (pure input, off the proj path)
            ao_loc = const.tile([128, NPB, TOK], fp8, tag="aoloc")
            xl = const.tile([128, NPB, TOK], bf16, tag="xl")
            for pb in range(NPB):
                eng = (nc.sync, nc.scalar)[pb % 2]
                eng.dma_start(xl[:, pb, :],
                              xloc_d.ap()[pb * 128:(pb + 1) * 128, :])

            # ======== Phase 2: causal attention, fp8 probs, DR PV ====
            a2a_in = [dram.tile([N_CORES, 128, TOKH], fp8, tag=f"a2a_in{b}",
                                name=f"a2a_in{b}")
                      for b in range(B)]
            a2a_out = [dram.tile([N_CORES, 128, TOKH], fp8, tag=f"a2a_out{b}",
                                 name=f"a2a_out{b}")
                       for b in range(B)]
            with (
                nc.named_scope("attn"),
                tc.tile_pool(name="p2e0", bufs=2) as p2e0,
                tc.tile_pool(name="p2e1", bufs=2) as p2e1,
                tc.tile_pool(name="p2s", bufs=2) as p2s,
                tc.tile_pool(name="ps_sc0", bufs=2, space="PSUM") as ps_sc0,
                tc.tile_pool(name="ps_sc1", bufs=1, space="PSUM") as ps_sc1,
                tc.tile_pool(name="ps_o", bufs=1, space="PSUM") as ps_o,
            ):
                ps_sc = (ps_sc0, ps_sc1)
                p2e = (p2e0, p2e1)

                def emit_pv(pv):
                    (b, pr, npr, fs0, ex_t, o_ps) = pv
                    for hh in range(H_LOC):
                        mmdr(o_ps[hh][0:65, fs0:512],
                             v_st[:, b * NPAIR + pr, hh, :, 0:65],
                             ex_t[hh][:, :, fs0:512],
                             start=(pr == 0), stop=(pr == npr - 1))

                def emit_norm(nrm):
                    (b, qo, o_ps) = nrm
                    r_row = []
                    for hh in range(H_LOC):
                        r = p2s.tile([1, 512], f32r, tag=f"r{hh}",
                                     name=f"r{hh}")
                        nc.vector.tensor_copy(r[:], o_ps[hh][64:65, :])
                        r_row.append(r)
                    rb_ps = [ps_sc1.tile([128, 2, 512], f32, tag="sc1",
                                         name=f"rb{hh}")
                             for hh in range(H_LOC)]
                    for hh in range(H_LOC):
                        mmr(rb_ps[hh][:, 0, :], ones_row_fr[:], r_row[hh][:],
                            start=True, stop=True)
                    for hh in range(H_LOC):
                        hp = hh * 64
                        rb_sb = p2s.tile([64, 512], f32, tag=f"rbsb{hh}",
                                         name=f"rbsb{hh}")
                        nc.vector.reciprocal_approx_fast(
                            rb_sb[:], rb_ps[hh][0:64, 0, :])
                        ao_t = p2s.tile([64, 512], fp8, tag=f"ao{hh}",
                                        name=f"ao{hh}")
                        nc.vector.tensor_mul(ao_t[:], o_ps[hh][0:64, :],
                                             rb_sb[:])
                        for jj in range(2):
                            a0 = qo // TOKH + jj
                            nc.sync.dma_start(
                                a2a_in[b][a0, hp:hp + 64, :],
                                ao_t[:, jj * TOKH:(jj + 1) * TOKH])

                # software pipeline: PV trails its sc/exp by one pair-unit
                # and the per-qc norm trails into the next qc, so the ACT
                # exp stream never waits on the PE and vice versa.
                pend_pv = None
                pend_norm = None
                for b in range(B):
                    for qc in range(NQC):
                        qo = qc * 512
                        nkb = qo // 128 + 4
                        npr = nkb // 2
                        o_ps = [ps_o.tile([128, 512], f32, tag=f"o{hh}",
                                          name=f"o{hh}")
                                for hh in range(H_LOC)]
                        for pr in range(npr):
                            kb0 = 2 * pr
                            diag = kb0 >= nkb - 4
                            fs0 = max(0, kb0 * 128 - qo) if diag else 0
                            sc_t = [ps_sc[hh].tile([128, 2, 512], f32,
                                                   tag=f"sc{hh}",
                                                   name=f"sc{hh}")
                                    for hh in range(H_LOC)]
                            for j in range(2):
                                kb = kb0 + j
                                fsj = max(0, kb * 128 - qo) if diag else 0
                                for hh in range(H_LOC):
                                    hp = hh * 64
                                    mm(sc_t[hh][:, j, fsj:512],
                                       kT_st[hp:hp + 64,
                                             b * TT + kb * 128:
                                             b * TT + (kb + 1) * 128],
                                       qT_st[hp:hp + 64,
                                             b * TT + qo + fsj:
                                             b * TT + qo + 512],
                                       start=True, stop=True)
                            ex_t = []
                            for hh in range(H_LOC):
                                ex = p2e[hh].tile([128, 2, 512], fp8,
                                                  tag=f"ex{hh}",
                                                  name=f"ex{hh}")
                                if not diag:
                                    nc.scalar.activation(
                                        ex[:], sc_t[hh][:], ACT.Exp,
                                        bias=ebias_col[:, 0:1],
                                        scale=1.0 / (SQ * SK))
                                else:
                                    for j in range(2):
                                        dj = (kb0 + j) * 128 - qo
                                        if dj > fs0:
                                            nc.vector.memset(
                                                ex[:, j, fs0:dj], 0.0)
                                        nc.vector.tensor_add(
                                            sc_t[hh][:, j, dj:dj + 128],
                                            sc_t[hh][:, j, dj:dj + 128],
                                            maskNeg[:])
                                        nc.scalar.activation(
                                            ex[:, j, dj:512],
                                            sc_t[hh][:, j, dj:512], ACT.Exp,
                                            bias=ebias_col[:, 0:1],
                                            scale=1.0 / (SQ * SK))
                                ex_t.append(ex)
                            if pend_pv is not None:
                                emit_pv(pend_pv)
                            if pend_norm is not None:
                                emit_norm(pend_norm)
                                pend_norm = None
                            pend_pv = (b, pr, npr, fs0, ex_t, o_ps)
                        pend_norm = (b, qo, o_ps)
                    # flush before firing this batch's AllToAll
                    emit_pv(pend_pv)
                    pend_pv = None
                    emit_norm(pend_norm)
                    pend_norm = None
                    nc.gpsimd.collective_compute(
                        "AllToAll", mybir.AluOpType.bypass,
                        replica_groups=[list(range(N_CORES))],
                        ins=[a2a_in[b].opt()], outs=[a2a_out[b].opt()])

            es_qkv.close()   # free q/k/v stores

            # ======== Phase 3: per-half proj + LN2 + FFN ====
            with (
                nc.named_scope("ffn"),
                tc.tile_pool(name="p3a", bufs=1) as p3a,
                tc.tile_pool(name="p3t", bufs=2) as p3t,
                tc.tile_pool(name="p3s", bufs=1) as p3s,
                tc.tile_pool(name="ps_pj", bufs=2, space="PSUM") as ps_pj,
                tc.tile_pool(name="ps_st", bufs=1, space="PSUM") as ps_st,
                tc.tile_pool(name="ps_f1", bufs=2, space="PSUM") as ps_f1,
                tc.tile_pool(name="ps_f2", bufs=2, space="PSUM") as ps_f2,
            ):
                y = p3a.tile([128, NPB, TOK], f32r, tag="y")
                yp = p3a.tile([128, NPB, TOK], bf16, tag="yp")

                for bb in range(B):
                    hs_ = slice(bb * TOKH, (bb + 1) * TOKH)
                    for a in range(N_CORES):
                        if bb == 0:
                            eng = (nc.sync, nc.scalar)[a % 2]
                        else:
                            eng = nc.gpsimd
                        eng.dma_start(ao_loc[:, a, hs_], a2a_out[bb][a, :, :])
                    # ---- proj (fp8 DR) + residual ----
                    for co in range(NPB):
                        pj_ps = ps_f1.tile([128, TOKH], f32, tag="f1",
                                           name="pj_ps")
                        for j in range(NPB // 2):
                            sl = slice(2 * j, 2 * j + 2)
                            mmdr(pj_ps[:],
                                 wproj_t[:, sl, co * 128:(co + 1) * 128],
                                 ao_loc[:, sl, hs_],
                                 start=(j == 0), stop=(j == NPB // 2 - 1))
                        # y = pj/SP + (x + bproj)
                        nc.vector.scalar_tensor_tensor(
                            out=y[:, co, hs_], in0=pj_ps[:],
                            scalar=inv_sp_col[:, 0:1],
                            in1=xl[:, co, hs_], op0=AOp.mult, op1=AOp.add)
                    # ---- LN2 stats ----
                    s_ps = ps_st.tile([1, TOKH], f32, tag="s")
                    s2_ps = ps_st.tile([1, TOKH], f32, tag="s2")
                    for pb in range(NPB):
                        sq = p3t.tile([128, TOKH], f32r, tag="sq")
                        nc.vector.tensor_mul(sq[:], y[:, pb, hs_],
                                             y[:, pb, hs_])
                        mmr(s_ps[:], ones_col_fr[:], y[:, pb, hs_],
                            start=(pb == 0), stop=(pb == NPB - 1))
                        mmr(s2_ps[:], ones_col_fr[:], sq[:],
                            start=(pb == 0), stop=(pb == NPB - 1))
                    mu = p3s.tile([1, TOKH], f32r, tag="mu")
                    e2 = p3s.tile([1, TOKH], f32r, tag="e2")
                    nc.scalar.mul(mu[:], s_ps[:], 1.0 / C)
                    nc.scalar.mul(e2[:], s2_ps[:], 1.0 / C)
                    var = p3s.tile([1, TOKH], f32r, tag="var")
                    nc.vector.tensor_mul(var[:], mu[:], mu[:])
                    nc.vector.tensor_sub(var[:], e2[:], var[:])
                    R2_ps = ps_pj.tile([128, TOKH], f32, tag="pj",
                                       name="R2_ps")
                    mmr(R2_ps[:], ones_row_fr[:], var[:], start=True, stop=True)
                    R2_std = p3s.tile([128, TOKH], f32, tag="R2std")
                    nc.scalar.activation(R2_std[:], R2_ps[:], ACT.Sqrt,
                                         bias=eps_col[:])
                    R2_sb = p3s.tile([128, TOKH], f32, tag="R2sb")
                    nc.vector.reciprocal_approx_fast(R2_sb[:], R2_std[:])
                    # broadcast mu*rstd, then yp = y*rstd - bcast(mu*rstd)
                    mr = p3s.tile([1, TOKH], f32r, tag="mr")
                    nc.vector.tensor_mul(mr[:], mu[:], R2_sb[0:1, :])
                    MR_ps = ps_pj.tile([128, TOKH], f32, tag="pj", name="MR_ps")
                    mmr(MR_ps[:], ones_row_fr[:], mr[:], start=True, stop=True)
                    MR_sb = p3s.tile([128, TOKH], f32, tag="MRsb")
                    nc.vector.tensor_copy(MR_sb[:], MR_ps[:])
                    for pb in range(NPB):
                        t = p3t.tile([128, TOKH], f32r, tag="t")
                        nc.vector.tensor_mul(t[:], y[:, pb, hs_], R2_sb[:])
                        nc.vector.tensor_sub(yp[:, pb, hs_], t[:], MR_sb[:])
                    # ---- FF1 (+ReLU with fused bias row) ----
                    F = p3a.tile([128, NHB, TOKH], bf16, tag="F")
                    for hb in range(NHB):
                        f1_ps = ps_f1.tile([128, TOKH], f32, tag="f1")
                        for pb in range(NPB):
                            mm(f1_ps[:], w1_t[:, pb, hb * 128:(hb + 1) * 128],
                               yp[:, pb, hs_],
                               start=(pb == 0), stop=(pb == NPB - 1))
                        nc.scalar.activation(F[:, hb, :], f1_ps[:], ACT.Relu,
                                             bias=fbias_t[:, hb:hb + 1])
                    # ---- FF2 + residual ----
                    for co in range(NPB):
                        f2_ps = ps_f2.tile([128, TOKH], f32, tag="f2")
                        for hb in range(NHB):
                            mm(f2_ps[:], w2_t[:, hb, co * 128:(co + 1) * 128],
                               F[:, hb, :],
                               start=(hb == 0), stop=(hb == NHB - 1))
                        ob = p3t.tile([128, TOKH], bf16, tag="ob")
                        nc.vector.scalar_tensor_tensor(
                            out=ob[:], in0=f2_ps[:],
                            scalar=bff2_t[:, co:co + 1],
                            in1=y[:, co, hs_], op0=AOp.add, op1=AOp.add)
                        nc.gpsimd.dma_start(
                            out_d.ap()[co * 128:(co + 1) * 128, hs_], ob[:])

    nc.compile()
    return nc


def _make_in_maps(x, Wq, Wk, Wv, Wproj, bproj, g1, b1, g2, b2,
                  W_ff1, b_ff1, W_ff2, b_ff2, TT=T):
    import ml_dtypes
    bf16 = ml_dtypes.bfloat16
    fp8 = ml_dtypes.float8_e4m3
    BT = B * TT
    TOK = BT // N_CORES
    TOKH = TOK // 2
    NCH = BT // 512
    NPB = C // 128
    NHB = FF // 128
    f = np.float32

    def shuf(w, nblk):
        """[nblk*128, M] row-major -> [128, nblk, M] partition-major."""
        w = np.asarray(w)
        return np.ascontiguousarray(
            w.reshape(nblk, 128, w.shape[1]).transpose(1, 0, 2))

    x2d = np.asarray(x, f).reshape(BT, C)
    # LN1 applied on the host (pure function of the input x); the
    # (x + bproj) residual flows through the separate xloc input
    mu = x2d.mean(1, keepdims=True)
    rstd = 1.0 / np.sqrt(x2d.var(1, keepdims=True) + EPS)
    h1 = ((x2d - mu) * rstd * np.asarray(g1, f) + np.asarray(b1, f)).astype(f)
    # xt pre-shuffled: [128, NCH, NPB, 512], chunk-contiguous per partition
    xts = np.ascontiguousarray(
        h1.T.reshape(NPB, 128, NCH, 512).transpose(1, 2, 0, 3)).astype(fp8)
    xraw = np.ascontiguousarray(
        (x2d + np.asarray(bproj, f)[None, :]).T).astype(bf16)
    w1f = shuf((np.asarray(g2, f)[:, None]
                * np.asarray(W_ff1, f)).astype(bf16), NPB)
    fbias = np.ascontiguousarray(
        (np.asarray(b2, f) @ np.asarray(W_ff1, f)
         + np.asarray(b_ff1, f)).astype(f).reshape(NHB, 128).T)
    w2f = shuf(np.asarray(W_ff2, f).astype(bf16), NHB)
    wpj = shuf((np.asarray(Wproj, f) * SP).astype(fp8), NPB)
    bf2 = np.ascontiguousarray(
        np.asarray(b_ff2, f).reshape(NPB, 128).T)

    in_maps = []
    for c in range(N_CORES):
        h0 = c * H_LOC
        per_head = []
        for W, s_ in ((Wq, SCALE * SQ), (Wk, SK), (Wv, SV)):
            wl = np.ascontiguousarray(
                np.transpose(np.asarray(W, f)[h0:h0 + H_LOC], (1, 0, 2))
            ).reshape(C, H_LOC * HS) * s_
            per_head.append(shuf(wl.astype(fp8), NPB))
        # split-token ownership: core c owns tokens [TOKH*c, TOKH*(c+1))
        # of EACH batch (matches the per-batch AllToAlls)
        cols = np.concatenate([
            np.arange(TOKH * c, TOKH * (c + 1)),
            np.arange(TT + TOKH * c, TT + TOKH * (c + 1))])
        in_maps.append({
            "xt": xts,
            "xloc": np.ascontiguousarray(xraw[:, cols]),
            "wq": per_head[0], "wk": per_head[1], "wv": per_head[2],
            "wproj": wpj,
            "wff1": w1f,
            "fbias": fbias,
            "wff2": w2f,
            "bff2": bf2,
        })
    return in_maps


def _gather_out(shards, TT=T):
    """Assemble per-core [C, TOK] shards (split-token ownership) -> [C, BT]."""
    BT = B * TT
    TOK = BT // N_CORES
    TOKH = TOK // 2
    outT = np.empty((C, BT), np.float32)
    for c, sh in enumerate(shards):
        cols = np.concatenate([
            np.arange(TOKH * c, TOKH * (c + 1)),
            np.arange(TT + TOKH * c, TT + TOKH * (c + 1))])
        outT[:, cols] = sh
    return outT


def kernel(**inputs):
    from concourse.bass_utils import run_bass_kernel_spmd
    if "nc" not in _cache:
        _cache["nc"] = _build()
    nc = _cache["nc"]
    in_maps = _make_in_maps(**inputs)
    res = run_bass_kernel_spmd(nc, in_maps, list(range(N_CORES)),
                               trace=bool(int(os.environ.get("KERNEL_TRACE", "0"))))
    _cache["last_result"] = res
    shards = [np.asarray(res.results[c]["out"], np.float32)
              for c in range(N_CORES)]                      # each [C, TOK]
    outT = _gather_out(shards)
    return np.ascontiguousarray(outT.T).reshape(B, T, C)
